# revision 1
# baseline (speedup 1.0000x reference)
"""GNN message-passing layer (EquivariantMPLayer) on 8 Trainium2 NeuronCores.

Sharding: edges are sharded by destination-node range (dst // (N/8)) so each
core aggregates its own node range locally -- no collectives needed. Per core,
edges are sorted by dst and grouped into 128-node sub-windows; each window's
edge list is split into two streams by src < N/2 (dma_gather indices are
int16, so each gather table must have < 32768 rows) and padded to 128-edge
blocks. Per-window per-stream block counts are equalized across cores (max
over cores) so a single SPMD program serves all 8 cores.

Device pipeline per 128-edge block:
  - dma_gather of x[src] (from the lo/hi half table) and x[dst] (from the
    core-local node table), batched ~8k rows per call
  - PE transposes to feature-major
  - L1 matmul (mw1, K split 128 + 16 for host-precomputed RBF features)
  - Silu (ACT, fused bias, PSUM->SBUF)
  - L2 matmul -> per-edge messages (edge-major)
  - one-hot (DVE iota-compare vs window-relative dst) + scatter-matmul
    accumulating aggT[64, 128-node window] in PSUM
  - stream-lo: window partials parked in SBUF; stream-hi: combined flush
    with host-precomputed 1/max(cnt,1) and mb2*(cnt>0)
Then an update MLP + LayerNorm over the core's nodes, written row-major.
"""

import numpy as np

N = 50000
E = 800000
DIN = 64
DOUT = 64
NB = 16
MAX_RADIUS = 10.0
NCORES = 8
P = 128
GB = 32  # gather batch, in 128-edge blocks (4096 rows per dma_gather)

_prog_cache = {}


def _wrap_idx16(arr):
    """[nblocks, 128] int idx -> [128, nblocks*8] int16 (16-wrapped, x8)."""
    nb = arr.shape[0]
    t16 = np.transpose(arr.reshape(nb, 8, 16), (2, 0, 1)).reshape(16, nb * 8)
    return np.tile(t16.astype(np.int16), (8, 1)).copy()


# ---------------------------------------------------------------------------
# Host-side structure / metadata
# ---------------------------------------------------------------------------

def _build_host_data(x, edge_index, edge_len, mw1, mb1, mw2, mb2,
                     uw1, ub1, uw2, ub2, ln_g, ln_b,
                     n=N, ncores=NCORES):
    nloc = n // ncores
    nhalf = (n + 1) // 2
    nw = (nloc + P - 1) // P
    npad = nw * P

    src = np.asarray(edge_index[0], dtype=np.int64)
    dst = np.asarray(edge_index[1], dtype=np.int64)
    x = np.asarray(x, dtype=np.float32)
    el = np.asarray(edge_len, dtype=np.float32)[:, 0]

    centers = np.linspace(0.0, MAX_RADIUS, NB, dtype=np.float64)
    width = (centers[1] - centers[0]) * 0.5
    rbf_all = np.exp(-((el[:, None].astype(np.float64) - centers) ** 2)
                     / (2.0 * width ** 2)).astype(np.float32)  # [E, 16]

    core_of = dst // nloc
    per_core = []
    cnt_s = np.zeros((2, ncores, nw), dtype=np.int64)
    for c in range(ncores):
        eids = np.nonzero(core_of == c)[0]
        dloc = (dst[eids] - c * nloc).astype(np.int64)
        order = np.argsort(dloc, kind="stable")
        eids = eids[order]
        dloc = dloc[order]
        w_of = dloc // P
        hi = (src[eids] >= nhalf).astype(np.int64)
        for s in (0, 1):
            cnt_s[s, c] = np.bincount(w_of[hi == s], minlength=nw)
        per_core.append((eids, dloc, w_of, hi))

    # per-(stream, window) block counts, equalized across cores
    bws = np.maximum(1, (cnt_s.max(axis=1) + P - 1) // P)  # [2, nw]
    # pad each stream's total to a multiple of 4 (supertile granularity)
    for s in (0, 1):
        bws[s, -1] += (-int(bws[s].sum())) % 4
    b_lo = int(bws[0].sum())
    b_hi = int(bws[1].sum())
    btot = b_lo + b_hi
    epad = btot * P

    # global block list: stream-lo blocks (windows in order), then stream-hi
    block_window = []
    block_stream = []
    for s in (0, 1):
        for w in range(nw):
            block_window += [w] * int(bws[s, w])
            block_stream += [s] * int(bws[s, w])
    block_window = np.array(block_window)
    block_stream = np.array(block_stream)
    # block offsets per (stream, window)
    boff = {}
    pos = 0
    for s in (0, 1):
        for w in range(nw):
            boff[(s, w)] = pos
            pos += int(bws[s, w])

    in_maps = []
    for c in range(ncores):
        eids, dloc, w_of, hi = per_core[c]
        sidx = np.zeros((btot, P), dtype=np.int64)
        didx = np.zeros((btot, P), dtype=np.int64)
        dwrel_arr = np.full((btot, P), 999.0, dtype=np.float32)
        rbf_arr = np.zeros((btot, P, NB), dtype=np.float32)
        for s in (0, 1):
            for w in range(nw):
                sel = np.nonzero((w_of == w) & (hi == s))[0]
                k = len(sel)
                if not k:
                    continue
                ee = eids[sel]
                b0 = boff[(s, w)]
                bidx = b0 + np.arange(k) // P
                pidx = np.arange(k) % P
                sidx[bidx, pidx] = src[ee] - s * nhalf
                didx[bidx, pidx] = dloc[sel]
                dwrel_arr[bidx, pidx] = (dloc[sel] - w * P).astype(np.float32)
                rbf_arr[bidx, pidx] = rbf_all[ee]

        cnt_n = np.zeros(npad, dtype=np.float32)
        cnt_n[:nloc] = np.bincount(dloc, minlength=nloc).astype(np.float32)
        inv = 1.0 / np.maximum(cnt_n, 1.0)
        has = (cnt_n > 0).astype(np.float32)
        fmul = np.broadcast_to(inv[None, :], (DOUT, npad)).copy()
        fadd = (np.asarray(mb2, np.float32)[:, None] * has[None, :]).copy()

        xt_loc = np.zeros((DIN, npad), dtype=np.float32)
        xt_loc[:, :nloc] = x[c * nloc:(c + 1) * nloc].T

        def padbf(a):
            out = np.zeros((a.shape[0], 2 * DIN), dtype=np.float16)
            out = out.astype(np.dtype('bfloat16') if hasattr(np, 'bfloat16')
                             else out.dtype)
            import ml_dtypes
            out = np.zeros((a.shape[0], 2 * DIN), dtype=ml_dtypes.bfloat16)
            out[:, :DIN] = a.astype(ml_dtypes.bfloat16)
            return out

        import ml_dtypes
        bf16 = ml_dtypes.bfloat16
        m = {
            "xa": padbf(x[:nhalf]),
            "xb": padbf(x[nhalf:]),
            "xloc": padbf(x[c * nloc:(c + 1) * nloc]),
            "sidx_lo": _wrap_idx16(sidx[:b_lo]),
            "sidx_hi": _wrap_idx16(sidx[b_lo:]),
            "didx": _wrap_idx16(didx),
            "dwrelT": dwrel_arr.T.copy(),                    # [128, btot] f32
            "rbfT": rbf_arr.reshape(epad, NB).T.astype(bf16),  # [16, epad]
            "xTloc": xt_loc,
            "fmul": fmul,
            "fadd": fadd,
            "mw1_sd": np.asarray(mw1, np.float32)[:2 * DIN].astype(bf16),
            "mw1_r": np.asarray(mw1, np.float32)[2 * DIN:].astype(bf16),
            "mb1": np.asarray(mb1, np.float32).reshape(2 * DOUT, 1).copy(),
            "mw2": np.asarray(mw2, np.float32).astype(bf16),
            # upd layout is [agg; x] -> swap uw1 row blocks to match
            "uw1": np.concatenate([np.asarray(uw1, np.float32)[DIN:],
                                   np.asarray(uw1, np.float32)[:DIN]], axis=0),
            "ub1": np.asarray(ub1, np.float32).reshape(DOUT, 1).copy(),
            "uw2": np.asarray(uw2, np.float32),
            "ub2": np.asarray(ub2, np.float32).reshape(DOUT, 1).copy(),
            "lng": np.broadcast_to(np.asarray(ln_g, np.float32)[None, :],
                                   (P, DOUT)).copy(),
            "lnb": np.broadcast_to(np.asarray(ln_b, np.float32)[None, :],
                                   (P, DOUT)).copy(),
            "iota": np.broadcast_to(np.arange(P).astype(bf16)[None, :],
                                    (P, P)).copy(),
            "ident": np.eye(P, dtype=np.float32).astype(bf16),
            "identf": np.eye(P, dtype=np.float32),
        }
        in_maps.append(m)

    struct = dict(n=n, nhalf=nhalf, nloc=nloc, nw=nw, npad=npad,
                  b_lo=b_lo, b_hi=b_hi, btot=btot, epad=epad,
                  bws=tuple(tuple(int(v) for v in row) for row in bws),
                  block_window=tuple(int(v) for v in block_window),
                  block_stream=tuple(int(v) for v in block_stream))
    return struct, in_maps


# ---------------------------------------------------------------------------
# Device program
# ---------------------------------------------------------------------------

def _build_program(struct, use_silu=True, debug_dump=False):
    import concourse.bass as bass
    import concourse.mybir as mybir
    import concourse.tile as tile
    from concourse import bacc

    f32 = mybir.dt.float32
    bf = mybir.dt.bfloat16
    i16 = mybir.dt.int16
    n, nhalf, nloc, nw, npad = (struct["n"], struct["nhalf"], struct["nloc"],
                                struct["nw"], struct["npad"])
    b_lo, b_hi, btot, epad = (struct["b_lo"], struct["b_hi"],
                              struct["btot"], struct["epad"])
    block_window = struct["block_window"]
    block_stream = struct["block_stream"]

    # first/last block of each (stream, window)
    wfirst = {}
    wlast = {}
    for g, (w, s) in enumerate(zip(block_window, block_stream)):
        wfirst.setdefault((s, w), g)
        wlast[(s, w)] = g

    nc = bacc.Bacc("TRN2", target_bir_lowering=False, debug=False,
                   enable_asserts=False, num_devices=NCORES,
                   num_swdge_queues=4)

    xa_d = nc.dram_tensor("xa", [nhalf, 2 * DIN], bf, kind="ExternalInput")
    xb_d = nc.dram_tensor("xb", [n - nhalf, 2 * DIN], bf,
                          kind="ExternalInput")
    xloc_d = nc.dram_tensor("xloc", [nloc, 2 * DIN], bf, kind="ExternalInput")
    sidx_lo_d = nc.dram_tensor("sidx_lo", [P, b_lo * 8], i16,
                               kind="ExternalInput")
    sidx_hi_d = nc.dram_tensor("sidx_hi", [P, b_hi * 8], i16,
                               kind="ExternalInput")
    didx_d = nc.dram_tensor("didx", [P, btot * 8], i16, kind="ExternalInput")
    dwrelT_d = nc.dram_tensor("dwrelT", [P, btot], f32, kind="ExternalInput")
    rbfT_d = nc.dram_tensor("rbfT", [NB, epad], bf, kind="ExternalInput")
    xTloc_d = nc.dram_tensor("xTloc", [DIN, npad], f32, kind="ExternalInput")
    fmul_d = nc.dram_tensor("fmul", [DOUT, npad], f32, kind="ExternalInput")
    fadd_d = nc.dram_tensor("fadd", [DOUT, npad], f32, kind="ExternalInput")
    mw1_sd_d = nc.dram_tensor("mw1_sd", [2 * DIN, 2 * DOUT], bf,
                              kind="ExternalInput")
    mw1_r_d = nc.dram_tensor("mw1_r", [NB, 2 * DOUT], bf,
                             kind="ExternalInput")
    mb1_d = nc.dram_tensor("mb1", [2 * DOUT, 1], f32, kind="ExternalInput")
    mw2_d = nc.dram_tensor("mw2", [2 * DOUT, DOUT], bf, kind="ExternalInput")
    uw1_d = nc.dram_tensor("uw1", [DIN + DOUT, DOUT], f32,
                           kind="ExternalInput")
    ub1_d = nc.dram_tensor("ub1", [DOUT, 1], f32, kind="ExternalInput")
    uw2_d = nc.dram_tensor("uw2", [DOUT, DOUT], f32, kind="ExternalInput")
    ub2_d = nc.dram_tensor("ub2", [DOUT, 1], f32, kind="ExternalInput")
    lng_d = nc.dram_tensor("lng", [P, DOUT], f32, kind="ExternalInput")
    lnb_d = nc.dram_tensor("lnb", [P, DOUT], f32, kind="ExternalInput")
    iota_d = nc.dram_tensor("iota", [P, P], bf, kind="ExternalInput")
    ident_d = nc.dram_tensor("ident", [P, P], bf, kind="ExternalInput")
    identf_d = nc.dram_tensor("identf", [P, P], f32, kind="ExternalInput")
    out_d = nc.dram_tensor("out", [npad, DOUT], f32, kind="ExternalOutput")
    if debug_dump:
        dbg_xT_d = nc.dram_tensor("dbg_xT", [P, 512], f32,
                                  kind="ExternalOutput")
        dbg_hT_d = nc.dram_tensor("dbg_hT", [P, 512], f32,
                                  kind="ExternalOutput")
        dbg_msg_d = nc.dram_tensor("dbg_msg", [P, 4 * DOUT], f32,
                                   kind="ExternalOutput")
        dbg_oh_d = nc.dram_tensor("dbg_oh", [P, P], f32,
                                  kind="ExternalOutput")
        dbg_upd_d = nc.dram_tensor("dbg_upd", [P, npad], f32,
                                   kind="ExternalOutput")

    AX = mybir.AxisListType
    OP = mybir.AluOpType
    ACT = mybir.ActivationFunctionType

    with tile.TileContext(nc) as tc:
        with (
            tc.tile_pool(name="const", bufs=1) as cpool,
            tc.tile_pool(name="gath", bufs=3) as gpool,
            tc.tile_pool(name="work", bufs=4) as wpool,
            tc.tile_pool(name="oh", bufs=8) as opool,
            tc.tile_pool(name="flush", bufs=2) as fpool,
            tc.tile_pool(name="pt", bufs=2, space="PSUM") as pt_pool,
            tc.tile_pool(name="ph", bufs=2, space="PSUM") as ph_pool,
            tc.tile_pool(name="pm", bufs=2, space="PSUM") as pm_pool,
            tc.tile_pool(name="pa", bufs=2, space="PSUM") as pa_pool,
        ):
            def cload(dram, shape, dtype=f32):
                t = cpool.tile(shape, dtype, name=dram.name + "_t")
                nc.sync.dma_start(out=t[:], in_=dram[:])
                return t

            mw1_sd_t = cload(mw1_sd_d, [2 * DIN, 2 * DOUT], bf)
            mw1_r_t = cload(mw1_r_d, [NB, 2 * DOUT], bf)
            mb1_t = cload(mb1_d, [2 * DOUT, 1])
            mw2_t = cload(mw2_d, [2 * DOUT, DOUT], bf)
            uw1_t = cload(uw1_d, [DIN + DOUT, DOUT])
            ub1_t = cload(ub1_d, [DOUT, 1])
            uw2_t = cload(uw2_d, [DOUT, DOUT])
            ub2_t = cload(ub2_d, [DOUT, 1])
            lng_t = cload(lng_d, [P, DOUT])
            lnb_t = cload(lnb_d, [P, DOUT])
            iota_t = cload(iota_d, [P, P], bf)
            ident_t = cload(ident_d, [P, P], bf)
            identf_t = cload(identf_d, [P, P])
            sidx_lo_t = cload(sidx_lo_d, [P, b_lo * 8], i16)
            sidx_hi_t = cload(sidx_hi_d, [P, b_hi * 8], i16)
            didx_t = cload(didx_d, [P, btot * 8], i16)
            dwrelT_t = cload(dwrelT_d, [P, btot])

            eps_t = cpool.tile([P, 1], f32, name="eps_t")
            nc.vector.memset(eps_t[:], 1e-5)

            # stream-lo window partials
            agglo_t = cpool.tile([DOUT, npad], f32, name="agglo_t")
            # combined update-MLP input: rows 0:64 = aggT, rows 64:128 = xT
            upd_t = cpool.tile([P, npad], f32, name="upd_t")
            nc.sync.dma_start(out=upd_t[DOUT:P, :], in_=xTloc_d[:])

            pa_cur = {}

            def do_stream(s, g0s, g1s, src_tab, sidx_t, sidx_goff):
                for b0 in range(g0s, g1s, GB):
                    gb = min(GB, g1s - b0)
                    xg = gpool.tile([P, 2, GB, 2 * DIN], bf,
                                    name=f"xg_{s}_{b0}", tag="xg")
                    c0 = (b0 - sidx_goff) * 8
                    bi = (b0 - g0s) // GB
                    q0 = (2 * bi) % 4
                    nc.gpsimd.dma_gather(
                        out_ap=xg[:, 0, 0:gb, :], in_ap=src_tab,
                        idxs_ap=sidx_t[:, c0:c0 + gb * 8],
                        num_idxs=gb * P, num_idxs_reg=gb * P,
                        elem_size=2 * DIN, single_packet=False, queue_num=q0)
                    nc.gpsimd.dma_gather(
                        out_ap=xg[:, 1, 0:gb, :], in_ap=xloc_d[:],
                        idxs_ap=didx_t[:, b0 * 8:(b0 + gb) * 8],
                        num_idxs=gb * P, num_idxs_reg=gb * P,
                        elem_size=2 * DIN, single_packet=False,
                        queue_num=q0 + 1)

                    for st0 in range(0, gb, 4):
                        st_blocks = [b0 + st0 + j for j in range(4)]
                        dbg_here = debug_dump and st_blocks[0] == 0

                        pxT = pt_pool.tile([P, 512], bf, tag="pxT",
                                           name=f"pxT_{st_blocks[0]}")
                        xg_flat = xg[:].rearrange("p a g d -> p (a g d)")
                        RW = 2 * DIN  # padded row width
                        for j in range(4):
                            gj = st0 + j
                            # full [128,128] transpose of [zeropad|dst_j]
                            # (64 cols before a dst row are the previous
                            # row's zero padding), then overwrite
                            # partitions 0:64 with src_j.T
                            o1 = (GB + gj) * RW - DIN
                            nc.tensor.transpose(
                                out=pxT[:, j * P:(j + 1) * P],
                                in_=xg_flat[:, o1:o1 + 2 * DIN],
                                identity=ident_t[:])
                            nc.tensor.transpose(
                                out=pxT[0:DIN, j * P:(j + 1) * P],
                                in_=xg_flat[:, gj * RW:gj * RW + DIN],
                                identity=ident_t[:])
                        xT_sb = wpool.tile([P, 512], bf, tag="xT",
                                           name=f"xT_{st_blocks[0]}")
                        nc.scalar.copy(out=xT_sb[:], in_=pxT[:])
                        if dbg_here:
                            nc.sync.dma_start(out=dbg_xT_d[:], in_=xT_sb[:])

                        rbf_t = wpool.tile([NB, 512], bf, tag="rbf",
                                           name=f"rbf_{st_blocks[0]}")
                        e0 = st_blocks[0] * P
                        nc.sync.dma_start(out=rbf_t[:],
                                          in_=rbfT_d[:, e0:e0 + 512])

                        ph = ph_pool.tile([P, 512], f32, tag="ph",
                                          name=f"ph_{st_blocks[0]}")
                        nc.tensor.matmul(ph[:], mw1_sd_t[:], xT_sb[:],
                                         start=True, stop=False)
                        nc.tensor.matmul(ph[:], mw1_r_t[:], rbf_t[:],
                                         start=False, stop=True)

                        hT_sb = wpool.tile([P, 512], bf, tag="hT",
                                           name=f"hT_{st_blocks[0]}")
                        if use_silu:
                            nc.scalar.activation(out=hT_sb[:], in_=ph[:],
                                                 func=ACT.Silu,
                                                 bias=mb1_t[:, 0:1])
                        else:
                            sg = wpool.tile([P, 512], bf, tag="sg",
                                            name=f"sg_{st_blocks[0]}")
                            nc.scalar.activation(out=sg[:], in_=ph[:],
                                                 func=ACT.Sigmoid,
                                                 bias=mb1_t[:, 0:1])
                            nc.scalar.activation(out=hT_sb[:], in_=ph[:],
                                                 func=ACT.Identity,
                                                 bias=mb1_t[:, 0:1])
                            nc.vector.tensor_tensor(out=hT_sb[:],
                                                    in0=hT_sb[:],
                                                    in1=sg[:], op=OP.mult)
                        if dbg_here:
                            nc.sync.dma_start(out=dbg_hT_d[:], in_=hT_sb[:])

                        pm = pm_pool.tile([P, 4 * DOUT], f32, tag="pm",
                                          name=f"pm_{st_blocks[0]}")
                        for j in range(4):
                            nc.tensor.matmul(pm[:, j * DOUT:(j + 1) * DOUT],
                                             hT_sb[:, j * P:(j + 1) * P],
                                             mw2_t[:], start=True, stop=True)
                        msg_sb = wpool.tile([P, 4 * DOUT], bf, tag="msg",
                                            name=f"msg_{st_blocks[0]}")
                        nc.scalar.copy(out=msg_sb[:], in_=pm[:])
                        if dbg_here:
                            nc.sync.dma_start(out=dbg_msg_d[:], in_=msg_sb[:])

                        for j in range(4):
                            g = st_blocks[j]
                            w = block_window[g]
                            oh = opool.tile([P, P], bf, tag="oh",
                                            name=f"oh_{g}")
                            nc.any.tensor_scalar(
                                out=oh[:], in0=iota_t[:],
                                scalar1=dwrelT_t[:, g:g + 1], scalar2=None,
                                op0=OP.is_equal)
                            if dbg_here and j == 0:
                                nc.sync.dma_start(out=dbg_oh_d[:], in_=oh[:])
                            if g == wfirst[(s, w)]:
                                pa_cur[w] = pa_pool.tile(
                                    [DOUT, P], f32, tag="pa",
                                    name=f"pa_s{s}_w{w}")
                            nc.tensor.matmul(
                                pa_cur[w][:],
                                msg_sb[:, j * DOUT:(j + 1) * DOUT], oh[:],
                                start=(g == wfirst[(s, w)]),
                                stop=(g == wlast[(s, w)]),
                                skip_group_check=True)
                            if g != wlast[(s, w)]:
                                continue
                            wc = slice(w * P, (w + 1) * P)
                            if s == 0:
                                nc.vector.tensor_copy(out=agglo_t[:, wc],
                                                      in_=pa_cur[w][:])
                            else:
                                fm = fpool.tile([DOUT, P], f32, tag="fm",
                                                name=f"fm_{w}")
                                fa = fpool.tile([DOUT, P], f32, tag="fa",
                                                name=f"fa_{w}")
                                nc.sync.dma_start(out=fm[:],
                                                  in_=fmul_d[:, wc])
                                nc.sync.dma_start(out=fa[:],
                                                  in_=fadd_d[:, wc])
                                nc.vector.tensor_tensor(
                                    out=upd_t[0:DOUT, wc], in0=pa_cur[w][:],
                                    in1=agglo_t[:, wc], op=OP.add)
                                nc.vector.tensor_tensor(
                                    out=upd_t[0:DOUT, wc],
                                    in0=upd_t[0:DOUT, wc],
                                    in1=fm[:], op=OP.mult)
                                nc.vector.tensor_tensor(
                                    out=upd_t[0:DOUT, wc],
                                    in0=upd_t[0:DOUT, wc],
                                    in1=fa[:], op=OP.add)
                            del pa_cur[w]

            do_stream(0, 0, b_lo, xa_d[:], sidx_lo_t[:], 0)
            do_stream(1, b_lo, btot, xb_d[:], sidx_hi_t[:], b_lo)

            if debug_dump:
                nc.sync.dma_start(out=dbg_upd_d[:], in_=upd_t[:])

            # ----------------- update MLP + LayerNorm -----------------
            UT = 512
            for u0 in range(0, npad, UT):
                cw = min(UT, npad - u0)
                nj = cw // P
                pu = ph_pool.tile([P, 512], f32, tag="ph", name=f"pu_{u0}")
                nc.tensor.matmul(pu[0:DOUT, 0:cw], uw1_t[:],
                                 upd_t[:, u0:u0 + cw], start=True, stop=True)
                uh_sb = wpool.tile([DOUT, UT], f32, tag="uh", name=f"uh_{u0}")
                if use_silu:
                    nc.scalar.activation(out=uh_sb[:, 0:cw],
                                         in_=pu[0:DOUT, 0:cw],
                                         func=ACT.Silu, bias=ub1_t[:, 0:1])
                else:
                    sg2 = wpool.tile([DOUT, UT], f32, tag="sg2",
                                     name=f"sg2_{u0}")
                    nc.scalar.activation(out=sg2[:, 0:cw],
                                         in_=pu[0:DOUT, 0:cw],
                                         func=ACT.Sigmoid, bias=ub1_t[:, 0:1])
                    nc.scalar.activation(out=uh_sb[:, 0:cw],
                                         in_=pu[0:DOUT, 0:cw],
                                         func=ACT.Identity, bias=ub1_t[:, 0:1])
                    nc.vector.tensor_tensor(out=uh_sb[:, 0:cw],
                                            in0=uh_sb[:, 0:cw],
                                            in1=sg2[:, 0:cw], op=OP.mult)
                pz = pt_pool.tile([P, 512], f32, tag="pxT", name=f"pz_{u0}")
                nc.tensor.matmul(pz[0:DOUT, 0:cw], uw2_t[:], uh_sb[:, 0:cw],
                                 start=True, stop=True)
                zT_sb = wpool.tile([DOUT, UT], f32, tag="zT", name=f"zT_{u0}")
                nc.scalar.activation(out=zT_sb[:, 0:cw], in_=pz[0:DOUT, 0:cw],
                                     func=ACT.Identity, bias=ub2_t[:, 0:1])

                pz2 = pm_pool.tile([P, 4 * DOUT], f32, tag="pm",
                                   name=f"pz2_{u0}")
                for j in range(nj):
                    nc.tensor.transpose(
                        out=pz2[:, j * DOUT:(j + 1) * DOUT],
                        in_=zT_sb[:, j * P:(j + 1) * P],
                        identity=identf_t[0:DOUT, 0:DOUT])
                # LayerNorm on [128, nj, 64] (free-axis per-node)
                zc = wpool.tile([P, 4 * DOUT], f32, tag="zc", name=f"zc_{u0}")
                red = wpool.tile([P, 4], f32, tag="red", name=f"red_{u0}")
                red2 = wpool.tile([P, 4], f32, tag="red2", name=f"red2_{u0}")
                z3 = pz2[:, 0:nj * DOUT].rearrange("p (j d) -> p j d", d=DOUT)
                nc.vector.tensor_reduce(out=red[:, 0:nj], in_=z3, axis=AX.X,
                                        op=OP.add)
                nc.vector.tensor_scalar_mul(red[:, 0:nj], red[:, 0:nj],
                                            -1.0 / DOUT)
                zc3 = zc[:, 0:nj * DOUT].rearrange("p (j d) -> p j d", d=DOUT)
                nc.vector.tensor_tensor(
                    out=zc3, in0=z3,
                    in1=red[:, 0:nj, None].to_broadcast([P, nj, DOUT]),
                    op=OP.add)
                sq = wpool.tile([P, 4 * DOUT], f32, tag="sq", name=f"sq_{u0}")
                sq3 = sq[:, 0:nj * DOUT].rearrange("p (j d) -> p j d", d=DOUT)
                nc.vector.tensor_tensor(out=sq3, in0=zc3, in1=zc3, op=OP.mult)
                nc.vector.tensor_reduce(out=red2[:, 0:nj], in_=sq3, axis=AX.X,
                                        op=OP.add)
                sd = wpool.tile([P, 4], f32, tag="sd", name=f"sd_{u0}")
                nc.scalar.activation(out=sd[:, 0:nj], in_=red2[:, 0:nj],
                                     func=ACT.Sqrt, scale=1.0 / DOUT,
                                     bias=eps_t[:, 0:1])
                rs = wpool.tile([P, 4], f32, tag="rs", name=f"rs_{u0}")
                nc.vector.reciprocal(out=rs[:, 0:nj], in_=sd[:, 0:nj])
                zn = wpool.tile([P, 4 * DOUT], f32, tag="zn", name=f"zn_{u0}")
                zn3 = zn[:, 0:nj * DOUT].rearrange("p (j d) -> p j d", d=DOUT)
                nc.vector.tensor_tensor(
                    out=zn3, in0=zc3,
                    in1=rs[:, 0:nj, None].to_broadcast([P, nj, DOUT]),
                    op=OP.mult)
                for j in range(nj):
                    js = slice(j * DOUT, (j + 1) * DOUT)
                    nc.vector.tensor_tensor(out=zn[:, js], in0=zn[:, js],
                                            in1=lng_t[:], op=OP.mult)
                    nc.vector.tensor_tensor(out=zn[:, js], in0=zn[:, js],
                                            in1=lnb_t[:], op=OP.add)
                    r0 = u0 + j * P
                    nc.sync.dma_start(out=out_d[r0:r0 + P, :],
                                      in_=zn[:, js])

    nc.compile()
    return nc


# ---------------------------------------------------------------------------
# Entry point
# ---------------------------------------------------------------------------

last_results = None


def kernel(x, edge_index, edge_vec, edge_len,
           mw1, mb1, mw2, mb2, uw1, ub1, uw2, ub2, ln_g, ln_b):
    global last_results
    import os
    from concourse.bass_utils import run_bass_kernel_spmd

    struct, in_maps = _build_host_data(
        x, edge_index, edge_len, mw1, mb1, mw2, mb2,
        uw1, ub1, uw2, ub2, ln_g, ln_b)

    key = (struct["n"], struct["btot"], struct["bws"])
    if key not in _prog_cache:
        _prog_cache[key] = _build_program(
            struct, use_silu=os.environ.get("K_NO_SILU", "") == "")
    nc = _prog_cache[key]

    kw = {}
    if os.environ.get("K_TRACE", ""):
        import profile_shim
        profile_shim.install()
        kw = dict(trace=True, trace_cores=list(range(NCORES)),
                  tmpdir="/tmp/ntff_out")
    res = run_bass_kernel_spmd(nc, in_maps, core_ids=list(range(NCORES)), **kw)
    last_results = res
    nloc = struct["nloc"]
    out = np.concatenate([res.results[c]["out"][:nloc] for c in range(NCORES)],
                         axis=0)
    return out.astype(np.float32)



# revision 2
# speedup vs baseline: 2.7798x; 2.7798x over previous
"""GNN message-passing layer (EquivariantMPLayer) on 8 Trainium2 NeuronCores.

Sharding: edges are sharded by destination-node range (dst // (N/8)) so each
core aggregates its own node range locally -- no collectives needed. Per core,
edges are sorted by dst and grouped into 128-node windows; each window's edge
list is padded to 128-edge blocks. Per-window block counts are equalized
across cores (max over cores) so a single SPMD program serves all 8 cores.

The host pre-gathers x[src] and x[dst] for every edge slot into a single
feature-major stream xcatT [128, epad] (rows 0:64 = src feats, 64:128 = dst
feats, columns in device consumption order), so the device needs no gathers
and no transposes: it just streams sequential DMA.

Device pipeline per 512-edge group (4 blocks):
  - DMA xcatT / rbfT chunks
  - L1 matmul (mw1 over 128 gathered feats + 16 host-precomputed RBF feats)
  - Silu (ACT, fused bias, PSUM->SBUF, bf16)
  - L2 matmul -> per-edge messages (edge-major)
  - per 128-edge block: one-hot vs window-relative dst (DVE is_equal with a
    free-axis broadcast) + scatter-matmul accumulating aggT[64, 128-node
    window] in PSUM; window flush applies host-precomputed 1/max(cnt,1) and
    mb2*(cnt>0)
Then an update MLP + LayerNorm over the core's nodes, written row-major.
"""

import numpy as np

N = 50000
E = 800000
DIN = 64
DOUT = 64
NB = 16
MAX_RADIUS = 10.0
NCORES = 8
P = 128

_prog_cache = {}


# ---------------------------------------------------------------------------
# Host-side structure / metadata
# ---------------------------------------------------------------------------

def _build_host_data(x, edge_index, edge_len, mw1, mb1, mw2, mb2,
                     uw1, ub1, uw2, ub2, ln_g, ln_b,
                     n=N, ncores=NCORES):
    import ml_dtypes
    bf16 = ml_dtypes.bfloat16

    nloc = n // ncores
    nw = (nloc + P - 1) // P
    npad = nw * P

    src = np.asarray(edge_index[0], dtype=np.int64)
    dst = np.asarray(edge_index[1], dtype=np.int64)
    x = np.asarray(x, dtype=np.float32)
    el = np.asarray(edge_len, dtype=np.float32)[:, 0]

    centers = np.linspace(0.0, MAX_RADIUS, NB, dtype=np.float64)
    width = (centers[1] - centers[0]) * 0.5
    rbf_all = np.exp(-((el[:, None].astype(np.float64) - centers) ** 2)
                     / (2.0 * width ** 2)).astype(np.float32)  # [E, 16]

    core_of = dst // nloc
    per_core = []
    cnt_w = np.zeros((ncores, nw), dtype=np.int64)
    for c in range(ncores):
        eids = np.nonzero(core_of == c)[0]
        dloc = (dst[eids] - c * nloc).astype(np.int64)
        order = np.argsort(dloc, kind="stable")
        eids = eids[order]
        dloc = dloc[order]
        w_of = dloc // P
        cnt_w[c] = np.bincount(w_of, minlength=nw)
        per_core.append((eids, dloc, w_of))

    # per-window block counts, equalized across cores; total padded to x4
    bws = np.maximum(1, (cnt_w.max(axis=0) + P - 1) // P)  # [nw]
    bws[-1] += (-int(bws.sum())) % 4
    btot = int(bws.sum())
    epad = btot * P

    block_window = np.repeat(np.arange(nw), bws)
    boff = np.concatenate([[0], np.cumsum(bws)[:-1]])  # first block of window

    in_maps = []
    for c in range(ncores):
        eids, dloc, w_of = per_core[c]
        # slot index for each edge: window base + position within window
        win_start = np.concatenate([[0], np.cumsum(cnt_w[c])[:-1]])
        pos_in_w = np.arange(len(eids)) - win_start[w_of]
        slot = boff[w_of] * P + pos_in_w  # [e_c]

        xcat = np.zeros((epad, 2 * DIN), dtype=np.float32)
        xcat[slot, :DIN] = x[src[eids]]
        xcat[slot, DIN:] = x[dst[eids]]
        xcatT = np.ascontiguousarray(xcat.T).astype(bf16)

        rbf = np.zeros((epad, NB), dtype=np.float32)
        rbf[slot] = rbf_all[eids]
        rbfT = np.ascontiguousarray(rbf.T).astype(bf16)

        dwrel = np.full(epad, 999.0, dtype=np.float32)
        dwrel[slot] = (dloc - w_of * P).astype(np.float32)
        dwrelT = np.ascontiguousarray(
            dwrel.reshape(btot, P).T).astype(bf16)  # [128, btot]

        cnt_n = np.zeros(npad, dtype=np.float32)
        cnt_n[:nloc] = np.bincount(dloc, minlength=nloc).astype(np.float32)
        inv = 1.0 / np.maximum(cnt_n, 1.0)
        has = (cnt_n > 0).astype(np.float32)
        fmul = np.broadcast_to(inv[None, :], (DOUT, npad)).copy()
        fadd = (np.asarray(mb2, np.float32)[:, None] * has[None, :]).copy()

        xt_loc = np.zeros((DIN, npad), dtype=np.float32)
        xt_loc[:, :nloc] = x[c * nloc:(c + 1) * nloc].T

        m = {
            "xcatT": xcatT,
            "rbfT": rbfT,
            "dwrelT": dwrelT,
            "xTloc": xt_loc,
            "fmul": fmul,
            "fadd": fadd,
            "mw1_sd": np.asarray(mw1, np.float32)[:2 * DIN].astype(bf16),
            "mw1_r": np.asarray(mw1, np.float32)[2 * DIN:].astype(bf16),
            "mb1": np.asarray(mb1, np.float32).reshape(2 * DOUT, 1).copy(),
            "mw2": np.asarray(mw2, np.float32).astype(bf16),
            # upd layout is [agg; x] -> swap uw1 row blocks to match
            "uw1": np.concatenate([np.asarray(uw1, np.float32)[DIN:],
                                   np.asarray(uw1, np.float32)[:DIN]], axis=0),
            "ub1": np.asarray(ub1, np.float32).reshape(DOUT, 1).copy(),
            "uw2": np.asarray(uw2, np.float32),
            "ub2": np.asarray(ub2, np.float32).reshape(DOUT, 1).copy(),
            "lng": np.broadcast_to(np.asarray(ln_g, np.float32)[None, :],
                                   (P, DOUT)).copy(),
            "lnb": np.broadcast_to(np.asarray(ln_b, np.float32)[None, :],
                                   (P, DOUT)).copy(),
            "iota": np.broadcast_to(
                np.arange(P, dtype=np.float32)[None, :].astype(bf16),
                (P, P)).copy(),
            "identf": np.eye(P, dtype=np.float32),
        }
        in_maps.append(m)

    struct = dict(n=n, nloc=nloc, nw=nw, npad=npad, btot=btot, epad=epad,
                  bws=tuple(int(v) for v in bws),
                  block_window=tuple(int(v) for v in block_window))
    return struct, in_maps


# ---------------------------------------------------------------------------
# Device program
# ---------------------------------------------------------------------------

def _build_program(struct):
    import concourse.bass as bass
    import concourse.mybir as mybir
    import concourse.tile as tile
    from concourse import bacc

    f32 = mybir.dt.float32
    bf = mybir.dt.bfloat16
    n, nloc, nw, npad = (struct["n"], struct["nloc"], struct["nw"],
                         struct["npad"])
    btot, epad = struct["btot"], struct["epad"]
    block_window = struct["block_window"]

    # first/last block of each window
    wfirst = {}
    wlast = {}
    for g, w in enumerate(block_window):
        wfirst.setdefault(w, g)
        wlast[w] = g

    nc = bacc.Bacc("TRN2", target_bir_lowering=False, debug=False,
                   enable_asserts=False, num_devices=NCORES)

    xcatT_d = nc.dram_tensor("xcatT", [P, epad], bf, kind="ExternalInput")
    rbfT_d = nc.dram_tensor("rbfT", [NB, epad], bf, kind="ExternalInput")
    dwrelT_d = nc.dram_tensor("dwrelT", [P, btot], bf, kind="ExternalInput")
    xTloc_d = nc.dram_tensor("xTloc", [DIN, npad], f32, kind="ExternalInput")
    fmul_d = nc.dram_tensor("fmul", [DOUT, npad], f32, kind="ExternalInput")
    fadd_d = nc.dram_tensor("fadd", [DOUT, npad], f32, kind="ExternalInput")
    mw1_sd_d = nc.dram_tensor("mw1_sd", [2 * DIN, 2 * DOUT], bf,
                              kind="ExternalInput")
    mw1_r_d = nc.dram_tensor("mw1_r", [NB, 2 * DOUT], bf,
                             kind="ExternalInput")
    mb1_d = nc.dram_tensor("mb1", [2 * DOUT, 1], f32, kind="ExternalInput")
    mw2_d = nc.dram_tensor("mw2", [2 * DOUT, DOUT], bf, kind="ExternalInput")
    uw1_d = nc.dram_tensor("uw1", [DIN + DOUT, DOUT], f32,
                           kind="ExternalInput")
    ub1_d = nc.dram_tensor("ub1", [DOUT, 1], f32, kind="ExternalInput")
    uw2_d = nc.dram_tensor("uw2", [DOUT, DOUT], f32, kind="ExternalInput")
    ub2_d = nc.dram_tensor("ub2", [DOUT, 1], f32, kind="ExternalInput")
    lng_d = nc.dram_tensor("lng", [P, DOUT], f32, kind="ExternalInput")
    lnb_d = nc.dram_tensor("lnb", [P, DOUT], f32, kind="ExternalInput")
    iota_d = nc.dram_tensor("iota", [P, P], bf, kind="ExternalInput")
    identf_d = nc.dram_tensor("identf", [P, P], f32, kind="ExternalInput")
    out_d = nc.dram_tensor("out", [npad, DOUT], f32, kind="ExternalOutput")

    AX = mybir.AxisListType
    OP = mybir.AluOpType
    ACT = mybir.ActivationFunctionType

    with tile.TileContext(nc) as tc:
        with (
            tc.tile_pool(name="const", bufs=1) as cpool,
            tc.tile_pool(name="gath", bufs=3) as gpool,
            tc.tile_pool(name="work", bufs=4) as wpool,
            tc.tile_pool(name="oh", bufs=8) as opool,
            tc.tile_pool(name="pt", bufs=2, space="PSUM") as pt_pool,
            tc.tile_pool(name="ph", bufs=2, space="PSUM") as ph_pool,
            tc.tile_pool(name="pm", bufs=2, space="PSUM") as pm_pool,
            tc.tile_pool(name="pa", bufs=2, space="PSUM") as pa_pool,
        ):
            def cload(dram, shape, dtype=f32):
                t = cpool.tile(shape, dtype, name=dram.name + "_t")
                nc.sync.dma_start(out=t[:], in_=dram[:])
                return t

            mw1_sd_t = cload(mw1_sd_d, [2 * DIN, 2 * DOUT], bf)
            mw1_r_t = cload(mw1_r_d, [NB, 2 * DOUT], bf)
            mb1_t = cload(mb1_d, [2 * DOUT, 1])
            mw2_t = cload(mw2_d, [2 * DOUT, DOUT], bf)
            uw1_t = cload(uw1_d, [DIN + DOUT, DOUT])
            ub1_t = cload(ub1_d, [DOUT, 1])
            uw2_t = cload(uw2_d, [DOUT, DOUT])
            ub2_t = cload(ub2_d, [DOUT, 1])
            lng_t = cload(lng_d, [P, DOUT])
            lnb_t = cload(lnb_d, [P, DOUT])
            iota_t = cload(iota_d, [P, P], bf)
            identf_t = cload(identf_d, [P, P])
            dwrelT_t = cload(dwrelT_d, [P, btot], bf)
            fmul_t = cload(fmul_d, [DOUT, npad])
            fadd_t = cload(fadd_d, [DOUT, npad])

            eps_t = cpool.tile([P, 1], f32, name="eps_t")
            nc.vector.memset(eps_t[:], 1e-5)

            # combined update-MLP input: rows 0:64 = aggT, rows 64:128 = xT
            upd_t = cpool.tile([P, npad], f32, name="upd_t")
            nc.sync.dma_start(out=upd_t[DOUT:P, :], in_=xTloc_d[:])

            pa_cur = {}

            for g0 in range(0, btot, 4):
                e0 = g0 * P
                xc = gpool.tile([P, 4 * P], bf, tag="xc", name=f"xc_{g0}")
                nc.sync.dma_start(out=xc[:], in_=xcatT_d[:, e0:e0 + 4 * P])
                rb = gpool.tile([NB, 4 * P], bf, tag="rb", name=f"rb_{g0}")
                nc.sync.dma_start(out=rb[:], in_=rbfT_d[:, e0:e0 + 4 * P])

                ph = ph_pool.tile([P, 4 * P], f32, tag="ph", name=f"ph_{g0}")
                nc.tensor.matmul(ph[:], mw1_sd_t[:], xc[:],
                                 start=True, stop=False)
                nc.tensor.matmul(ph[:], mw1_r_t[:], rb[:],
                                 start=False, stop=True)

                hT = wpool.tile([P, 4 * P], bf, tag="hT", name=f"hT_{g0}")
                nc.scalar.activation(out=hT[:], in_=ph[:],
                                     func=ACT.Silu, bias=mb1_t[:, 0:1])

                pm = pm_pool.tile([P, 4 * DOUT], f32, tag="pm",
                                  name=f"pm_{g0}")
                for j in range(4):
                    nc.tensor.matmul(pm[:, j * DOUT:(j + 1) * DOUT],
                                     hT[:, j * P:(j + 1) * P],
                                     mw2_t[:], start=True, stop=True)
                msg = wpool.tile([P, 4 * DOUT], bf, tag="msg",
                                 name=f"msg_{g0}")
                nc.scalar.copy(out=msg[:], in_=pm[:])

                for j in range(4):
                    g = g0 + j
                    w = block_window[g]
                    oh = opool.tile([P, P], bf, tag="oh", name=f"oh_{g}")
                    nc.vector.tensor_tensor(
                        out=oh[:], in0=iota_t[:],
                        in1=dwrelT_t[:, g:g + 1].to_broadcast((P, P)),
                        op=OP.is_equal)
                    if g == wfirst[w]:
                        pa_cur[w] = pa_pool.tile([DOUT, P], f32, tag="pa",
                                                 name=f"pa_w{w}")
                    nc.tensor.matmul(
                        pa_cur[w][:],
                        msg[:, j * DOUT:(j + 1) * DOUT], oh[:],
                        start=(g == wfirst[w]),
                        stop=(g == wlast[w]),
                        skip_group_check=True)
                    if g != wlast[w]:
                        continue
                    wc = slice(w * P, (w + 1) * P)
                    nc.vector.tensor_tensor(
                        out=upd_t[0:DOUT, wc], in0=pa_cur[w][:],
                        in1=fmul_t[:, wc], op=OP.mult)
                    nc.vector.tensor_tensor(
                        out=upd_t[0:DOUT, wc], in0=upd_t[0:DOUT, wc],
                        in1=fadd_t[:, wc], op=OP.add)
                    del pa_cur[w]

            # ----------------- update MLP + LayerNorm -----------------
            UT = 512
            for u0 in range(0, npad, UT):
                cw = min(UT, npad - u0)
                nj = cw // P
                pu = ph_pool.tile([P, 512], f32, tag="ph", name=f"pu_{u0}")
                nc.tensor.matmul(pu[0:DOUT, 0:cw], uw1_t[:],
                                 upd_t[:, u0:u0 + cw], start=True, stop=True)
                uh_sb = wpool.tile([DOUT, UT], f32, tag="uh", name=f"uh_{u0}")
                nc.scalar.activation(out=uh_sb[:, 0:cw],
                                     in_=pu[0:DOUT, 0:cw],
                                     func=ACT.Silu, bias=ub1_t[:, 0:1])
                pz = pt_pool.tile([P, 512], f32, tag="pz", name=f"pz_{u0}")
                nc.tensor.matmul(pz[0:DOUT, 0:cw], uw2_t[:], uh_sb[:, 0:cw],
                                 start=True, stop=True)
                zT_sb = wpool.tile([DOUT, UT], f32, tag="zT", name=f"zT_{u0}")
                nc.scalar.activation(out=zT_sb[:, 0:cw], in_=pz[0:DOUT, 0:cw],
                                     func=ACT.Identity, bias=ub2_t[:, 0:1])

                pz2 = pm_pool.tile([P, 4 * DOUT], f32, tag="pm",
                                   name=f"pz2_{u0}")
                for j in range(nj):
                    nc.tensor.transpose(
                        out=pz2[:, j * DOUT:(j + 1) * DOUT],
                        in_=zT_sb[:, j * P:(j + 1) * P],
                        identity=identf_t[0:DOUT, 0:DOUT])
                # LayerNorm on [128, nj, 64] (free-axis per-node)
                zc = wpool.tile([P, 4 * DOUT], f32, tag="zc", name=f"zc_{u0}")
                red = wpool.tile([P, 4], f32, tag="red", name=f"red_{u0}")
                red2 = wpool.tile([P, 4], f32, tag="red2", name=f"red2_{u0}")
                z3 = pz2[:, 0:nj * DOUT].rearrange("p (j d) -> p j d", d=DOUT)
                nc.vector.tensor_reduce(out=red[:, 0:nj], in_=z3, axis=AX.X,
                                        op=OP.add)
                nc.vector.tensor_scalar_mul(red[:, 0:nj], red[:, 0:nj],
                                            -1.0 / DOUT)
                zc3 = zc[:, 0:nj * DOUT].rearrange("p (j d) -> p j d", d=DOUT)
                nc.vector.tensor_tensor(
                    out=zc3, in0=z3,
                    in1=red[:, 0:nj, None].to_broadcast([P, nj, DOUT]),
                    op=OP.add)
                sq = wpool.tile([P, 4 * DOUT], f32, tag="sq", name=f"sq_{u0}")
                sq3 = sq[:, 0:nj * DOUT].rearrange("p (j d) -> p j d", d=DOUT)
                nc.vector.tensor_tensor(out=sq3, in0=zc3, in1=zc3, op=OP.mult)
                nc.vector.tensor_reduce(out=red2[:, 0:nj], in_=sq3, axis=AX.X,
                                        op=OP.add)
                sd = wpool.tile([P, 4], f32, tag="sd", name=f"sd_{u0}")
                nc.scalar.activation(out=sd[:, 0:nj], in_=red2[:, 0:nj],
                                     func=ACT.Sqrt, scale=1.0 / DOUT,
                                     bias=eps_t[:, 0:1])
                rs = wpool.tile([P, 4], f32, tag="rs", name=f"rs_{u0}")
                nc.vector.reciprocal(out=rs[:, 0:nj], in_=sd[:, 0:nj])
                zn = wpool.tile([P, 4 * DOUT], f32, tag="zn", name=f"zn_{u0}")
                zn3 = zn[:, 0:nj * DOUT].rearrange("p (j d) -> p j d", d=DOUT)
                nc.vector.tensor_tensor(
                    out=zn3, in0=zc3,
                    in1=rs[:, 0:nj, None].to_broadcast([P, nj, DOUT]),
                    op=OP.mult)
                for j in range(nj):
                    js = slice(j * DOUT, (j + 1) * DOUT)
                    nc.vector.tensor_tensor(out=zn[:, js], in0=zn[:, js],
                                            in1=lng_t[:], op=OP.mult)
                    nc.vector.tensor_tensor(out=zn[:, js], in0=zn[:, js],
                                            in1=lnb_t[:], op=OP.add)
                    r0 = u0 + j * P
                    nc.sync.dma_start(out=out_d[r0:r0 + P, :],
                                      in_=zn[:, js])

    nc.compile()
    return nc


# ---------------------------------------------------------------------------
# Entry point
# ---------------------------------------------------------------------------

last_results = None


def kernel(x, edge_index, edge_vec, edge_len,
           mw1, mb1, mw2, mb2, uw1, ub1, uw2, ub2, ln_g, ln_b):
    global last_results
    import os
    from concourse.bass_utils import run_bass_kernel_spmd

    struct, in_maps = _build_host_data(
        x, edge_index, edge_len, mw1, mb1, mw2, mb2,
        uw1, ub1, uw2, ub2, ln_g, ln_b)

    key = (struct["n"], struct["btot"], struct["bws"])
    if key not in _prog_cache:
        _prog_cache[key] = _build_program(struct)
    nc = _prog_cache[key]

    kw = {}
    if os.environ.get("K_TRACE", ""):
        import profile_shim
        profile_shim.install()
        kw = dict(trace=True, trace_cores=list(range(NCORES)),
                  tmpdir="/tmp/ntff_out")
    res = run_bass_kernel_spmd(nc, in_maps, core_ids=list(range(NCORES)), **kw)
    last_results = res
    nloc = struct["nloc"]
    out = np.concatenate([res.results[c]["out"][:nloc] for c in range(NCORES)],
                         axis=0)
    return out.astype(np.float32)
